# revision 9
# baseline (speedup 1.0000x reference)
"""CrossSymmetricModal trn2 kernel v5: rank-4 factorization + branch-a-first
schedule.

Rank-4 structure (clinical is [B,1,L]): phi = (cli[m-1], cli[m], cli[m+1], 1).
- branch a: scores = g.phi (g = 4-row conv of image, weights folded on host),
  ctx collapses to U = phi.et with den riding as ones-columns at partition 32,
  out conv contracts (tap, feature) via a padded [68,*] lhsT (blocks at
  partitions 0/32/64 to satisfy 32-partition alignment).
- branch b: scores = phi.h (h shares the fused 4-row conv with g); v/ctx/out
  conv stay dense.

Schedule: branch a's attention is exp(scalar)-gated, so branch b's v-convs are
interleaved into it as PE filler; AllReduce-a then hides under branch b's
dense compute, and only AllReduce-b is tail-exposed. finalize-a runs on
DVE+gpsimd mid-branch-b; finalize-b (scalar+DVE) is the tail.
"""
import os
import sys

sys.path.insert(0, '/opt/trn_rl_repo')

import ml_dtypes
import numpy as np

from concourse import bacc, mybir, tile
from concourse.bass_utils import run_bass_kernel_spmd

S = 2
NCORES = 8
C = 256
CT = 2
L = 1024
LS = 2
EPS = 1e-5
SCALE = 1.0 / 16.0
NSTAT = 16 * L

F32 = mybir.dt.float32
BF16 = mybir.dt.bfloat16
NPBF = ml_dtypes.bfloat16
AF = mybir.ActivationFunctionType
OP = mybir.AluOpType
AXX = mybir.AxisListType.X

_NC_CACHE = []


def _build_nc():
    nc = bacc.Bacc(num_devices=NCORES)

    imm_p = nc.declare_dram_parameter("imm", [S, CT, 128, L + 2], BF16, isOutput=False)
    cli_p = nc.declare_dram_parameter("cli", [S, 4, L], BF16, isOutput=False)
    clit_p = nc.declare_dram_parameter("clit", [S, 128, 288], BF16, isOutput=False)
    wgh_p = nc.declare_dram_parameter("wgh", [128, 216], BF16, isOutput=False)
    ghb_p = nc.declare_dram_parameter("ghb", [36, 1], F32, isOutput=False)
    owa_p = nc.declare_dram_parameter("owa", [68, 256], BF16, isOutput=False)
    wvb_p = nc.declare_dram_parameter("wvb", [CT, 3, 128, C], BF16, isOutput=False)
    vbb_p = nc.declare_dram_parameter("vbb", [128, C], F32, isOutput=False)
    wob_p = nc.declare_dram_parameter("wob", [CT, 128, 768], BF16, isOutput=False)
    bias_p = {}
    for name in ("oba", "obb", "ga_a", "be_a", "ga_b", "be_b"):
        bias_p[name] = nc.declare_dram_parameter(name, [128, CT], F32, isOutput=False)
    out_p = nc.declare_dram_parameter("out", [S, 2, CT, 128, L], F32, isOutput=True)

    from contextlib import ExitStack
    with tile.TileContext(nc) as tc, ExitStack() as es:
        ec = es.enter_context
        wgt = ec(tc.tile_pool(name="wgt", bufs=1))
        io = ec(tc.tile_pool(name="io", bufs=1))
        gs = ec(tc.tile_pool(name="gs", bufs=1))
        vtp = ec(tc.tile_pool(name="vtp", bufs=1))
        ex = ec(tc.tile_pool(name="ex", bufs=2))
        cx = ec(tc.tile_pool(name="cx", bufs=2))
        v3p = ec(tc.tile_pool(name="v3p", bufs=1))
        op_pool = ec(tc.tile_pool(name="op", bufs=1))
        sm = ec(tc.tile_pool(name="sm", bufs=2))
        sqp = ec(tc.tile_pool(name="sqp", bufs=2))
        st = ec(tc.tile_pool(name="st", bufs=1))
        bn = ec(tc.tile_pool(name="bn", bufs=3))
        dram = ec(tc.tile_pool(name="dram", bufs=1, space="DRAM"))
        ps_conv = ec(tc.tile_pool(name="psc", bufs=2, space="PSUM"))
        ps_sc = ec(tc.tile_pool(name="pss", bufs=3, space="PSUM"))
        ps_cu = ec(tc.tile_pool(name="psx", bufs=1, space="PSUM"))
        ps_den = ec(tc.tile_pool(name="psd", bufs=1, space="PSUM"))
        if True:
            # ---- DMAs: first conv's inputs first ----
            wgh_sb = wgt.tile([128, 216], BF16, tag="wgh")
            nc.sync.dma_start(out=wgh_sb, in_=wgh_p[:, :])
            ghb_sb = wgt.tile([36, 1], F32, tag="ghb")
            nc.sync.dma_start(out=ghb_sb, in_=ghb_p[:, :])
            imm = [[None] * CT for _ in range(S)]
            for kt in range(CT):
                t_ = io.tile([128, L + 2], BF16, tag=f"imm_0_{kt}", name=f"imm_0_{kt}")
                nc.sync.dma_start(out=t_, in_=imm_p[0, kt])
                imm[0][kt] = t_
            wvb_sb = []
            for kt in range(CT):
                row = []
                for t in range(3):
                    t_ = wgt.tile([128, C], BF16, tag=f"wvb_{kt}_{t}")
                    nc.sync.dma_start(out=t_, in_=wvb_p[kt, t])
                    row.append(t_)
                wvb_sb.append(row)
            for kt in range(CT):
                t_ = io.tile([128, L + 2], BF16, tag=f"imm_1_{kt}", name=f"imm_1_{kt}")
                nc.sync.dma_start(out=t_, in_=imm_p[1, kt])
                imm[1][kt] = t_
            vbb_sb = wgt.tile([128, C], F32, tag="vbb")
            nc.sync.dma_start(out=vbb_sb, in_=vbb_p[:, :])
            cli_sb = []
            clit_sb = []
            for s in range(S):
                t_ = io.tile([4, L], BF16, tag=f"cli_{s}")
                nc.sync.dma_start(out=t_, in_=cli_p[s])
                cli_sb.append(t_)
                t2 = io.tile([128, 288], BF16, tag=f"clit_{s}")
                nc.sync.dma_start(out=t2, in_=clit_p[s])
                clit_sb.append(t2)
            owa_sb = wgt.tile([68, 256], BF16, tag="owa")
            nc.sync.dma_start(out=owa_sb, in_=owa_p[:, :])
            bias = {}
            for name in ("oba", "ga_a", "be_a"):
                t_ = wgt.tile([128, CT], F32, tag=name)
                nc.sync.dma_start(out=t_, in_=bias_p[name][:, :])
                bias[name] = t_
            wob_sb = []
            for kt in range(CT):
                t_ = wgt.tile([128, 768], BF16, tag=f"wob_{kt}")
                nc.sync.dma_start(out=t_, in_=wob_p[kt])
                wob_sb.append(t_)
            for name in ("obb", "ga_b", "be_b"):
                t_ = wgt.tile([128, CT], F32, tag=name)
                nc.sync.dma_start(out=t_, in_=bias_p[name][:, :])
                bias[name] = t_

            # ---- constants ----
            ones_full = wgt.tile([128, 128], BF16, tag="ones_full")
            nc.vector.memset(ones_full, 1.0)
            eps_sb = wgt.tile([128, 1], F32, tag="eps_sb")
            nc.vector.memset(eps_sb, EPS)
            zero_col = wgt.tile([128, 1], BF16, tag="zero_col")
            nc.vector.memset(zero_col, 0.0)
            sqrt_warm = wgt.tile([128, 1], F32, tag="sqrt_warm")
            nc.scalar.activation(out=sqrt_warm, in_=eps_sb, func=AF.Sqrt)

            # ---- g/h fused conv: one PE pass makes both 4-row convs ----
            g_sb = []
            h_sb = []
            for s in range(S):
                g_ = gs.tile([4, L], BF16, tag=f"g_{s}", name=f"g_{s}")
                h_ = gs.tile([4, L], BF16, tag=f"h_{s}", name=f"h_{s}")
                g_sb.append(g_)
                h_sb.append(h_)

            def gh_conv(s):
                for ls in range(LS):
                    p = ps_conv.tile([128, 512], F32, tag="conv", name="convp")
                    n = 0
                    for kt in range(CT):
                        for t in range(3):
                            nc.tensor.matmul(
                                p[0:36],
                                lhsT=wgh_sb[:, (kt * 3 + t) * 36:(kt * 3 + t + 1) * 36],
                                rhs=imm[s][kt][:, ls * 512 + t: ls * 512 + t + 512],
                                start=(n == 0), stop=(n == 5))
                            n += 1
                    nc.scalar.activation(
                        out=g_sb[s][:, ls * 512:(ls + 1) * 512], in_=p[0:4],
                        func=AF.Identity, bias=ghb_sb[0:4, 0:1], scale=1.0)
                    nc.scalar.activation(
                        out=h_sb[s][:, ls * 512:(ls + 1) * 512], in_=p[32:36],
                        func=AF.Identity, bias=ghb_sb[32:36, 0:1], scale=1.0)

            # ---- branch b v-conv units (PE filler inside branch a) ----
            vt = {}

            def bv_unit(s, mt):
                p = ps_conv.tile([128, C], F32, tag="conv", name="convp")
                n = 0
                for kt in range(CT):
                    for t in range(3):
                        nc.tensor.matmul(
                            p,
                            lhsT=imm[s][kt][:, mt * 128 + t: mt * 128 + t + 128],
                            rhs=wvb_sb[kt][t],
                            start=(n == 0), stop=(n == 5))
                        n += 1
                v_ = vtp.tile([128, C], BF16, tag=f"vt{s}_{mt}", name=f"vt{s}_{mt}")
                nc.vector.tensor_add(out=v_, in0=p, in1=vbb_sb)
                vt[(s, mt)] = v_

            fillers = [(s, mt) for s in range(S) for mt in range(8)]
            fill_i = [0]

            def fill(k):
                while k > 0 and fill_i[0] < len(fillers):
                    s, mt = fillers[fill_i[0]]
                    bv_unit(s, mt)
                    fill_i[0] += 1
                    k -= 1

            o_tiles = {}
            slots = {}
            for br in range(2):
                slots[br] = st.tile([128, 4 * S * LS], F32, tag=f"slots{br}", name=f"slots{br}")

            def sq_stat(br, osl, j):
                sq = sqp.tile([128, 512], F32, tag="sq", name="sq")
                nc.vector.scalar_tensor_tensor(
                    out=sq, in0=osl, scalar=1.0, in1=osl,
                    op0=OP.mult, op1=OP.mult,
                    accum_out=slots[br][:, j:j + 1])

            # ---- branch a attention: U = phi.et with den columns ----
            def a_attention(s):
                v3 = v3p.tile([68, L + 2], BF16, tag=f"v3_{s}", name=f"v3_{s}")
                nc.vector.memset(v3, 0.0)
                for ls in range(LS):
                    ets = {}

                    def _sc_exp(mt):
                        sc = ps_sc.tile([128, 512], F32, tag="sc", name="sc")
                        nc.tensor.matmul(
                            sc, lhsT=cli_sb[s][:, mt * 128:(mt + 1) * 128],
                            rhs=g_sb[s][:, ls * 512:(ls + 1) * 512],
                            start=True, stop=True)
                        et = ex.tile([128, 512], BF16, tag=f"et{mt}", name="et")
                        nc.scalar.activation(out=et, in_=sc, func=AF.Exp, scale=SCALE)
                        ets[mt] = et

                    u_ps = ps_cu.tile([128, 512], F32, tag=f"ctxp{ls}", name="u_ps")
                    _sc_exp(0)
                    _sc_exp(1)
                    _sc_exp(2)
                    for mt in range(8):
                        if mt + 3 < 8:
                            _sc_exp(mt + 3)
                        et = ets.pop(mt)
                        nc.tensor.matmul(
                            u_ps[0:36], lhsT=clit_sb[s][:, mt * 36:(mt + 1) * 36],
                            rhs=et, start=(mt == 0), stop=(mt == 7))
                        fill(1)
                    den4 = sm.tile([4, 512], F32, tag="den4", name="den4")
                    nc.vector.tensor_copy(out=den4, in_=u_ps[32:36])
                    rec4 = sm.tile([4, 512], F32, tag="rec4", name="rec4")
                    nc.vector.reciprocal_approx_fast(out=rec4, in_=den4)
                    # V3 feature blocks at partitions 0/32/64; row 3 of each
                    # block is U[3]*rec = den/den = 1 (the folded v-bias lane)
                    for t in range(3):
                        c0 = ls * 512 + 2 - t
                        nc.vector.tensor_mul(
                            out=v3[32 * t:32 * t + 4, c0:c0 + 512],
                            in0=u_ps[0:4], in1=rec4)
                return v3

            def a_out_conv(s, v3):
                for ct in range(CT):
                    o_sb = op_pool.tile([128, L], F32, tag=f"o_0_{s}_{ct}", name=f"o_0_{s}_{ct}")
                    o_tiles[(0, s, ct)] = o_sb
                    for ls in range(LS):
                        p = ps_conv.tile([128, 512], F32, tag="conv", name="convp")
                        nc.tensor.matmul(
                            p, lhsT=owa_sb[:, ct * 128:(ct + 1) * 128],
                            rhs=v3[:, 1 + ls * 512: 1 + ls * 512 + 512],
                            start=True, stop=True)
                        osl = o_sb[:, ls * 512:(ls + 1) * 512]
                        i = ct * S * LS + s * LS + ls
                        nc.scalar.activation(
                            out=osl, in_=p, func=AF.Identity,
                            bias=bias["oba"][:, ct:ct + 1], scale=1.0,
                            accum_out=slots[0][:, i:i + 1])
                        sq_stat(0, osl, (2 + ct) * S * LS + s * LS + ls)

            # ---- branch b attention (dense v, rank-4 scores) ----
            def b_attention(s):
                ctx = [cx.tile([128, L + 2], BF16, tag=f"ctx{ct}", name=f"ctx{ct}") for ct in range(CT)]
                for ct in range(CT):
                    nc.vector.tensor_copy(out=ctx[ct][:, 0:1], in_=zero_col)
                    nc.vector.tensor_copy(out=ctx[ct][:, L + 1:L + 2], in_=zero_col)
                for ls in range(LS):
                    ets = {}

                    def _sc_exp(mt):
                        sc = ps_sc.tile([128, 512], F32, tag="sc", name="sc")
                        nc.tensor.matmul(
                            sc, lhsT=h_sb[s][:, mt * 128:(mt + 1) * 128],
                            rhs=cli_sb[s][:, ls * 512:(ls + 1) * 512],
                            start=True, stop=True)
                        et = ex.tile([128, 512], BF16, tag=f"et{mt}", name="et")
                        nc.scalar.activation(out=et, in_=sc, func=AF.Exp, scale=SCALE)
                        ets[mt] = et

                    ctx_ps = [ps_cu.tile([128, 512], F32, tag=f"ctxp{ct}", name=f"ctxp{ct}") for ct in range(CT)]
                    den_ps = ps_den.tile([128, 512], F32, tag="den", name="den_ps")
                    _sc_exp(0)
                    _sc_exp(1)
                    _sc_exp(2)
                    for mt in range(8):
                        if mt + 3 < 8:
                            _sc_exp(mt + 3)
                        et = ets.pop(mt)
                        for ct in range(CT):
                            nc.tensor.matmul(
                                ctx_ps[ct], lhsT=vt[(s, mt)][:, ct * 128:(ct + 1) * 128],
                                rhs=et, start=(mt == 0), stop=(mt == 7))
                        nc.tensor.matmul(
                            den_ps, lhsT=ones_full, rhs=et,
                            start=(mt == 0), stop=(mt == 7))
                    recip = sm.tile([128, 512], F32, tag="recip", name="recip")
                    nc.vector.reciprocal_approx_fast(out=recip, in_=den_ps)
                    for ct in range(CT):
                        nc.vector.tensor_mul(
                            out=ctx[ct][:, 1 + ls * 512: 1 + (ls + 1) * 512],
                            in0=ctx_ps[ct], in1=recip)
                return ctx

            def b_out_conv(s, ctx):
                for ct in range(CT):
                    o_sb = op_pool.tile([128, L], F32, tag=f"o_1_{s}_{ct}", name=f"o_1_{s}_{ct}")
                    o_tiles[(1, s, ct)] = o_sb
                    for ls in range(LS):
                        p = ps_conv.tile([128, 512], F32, tag="conv", name="convp")
                        n = 0
                        for kt in range(CT):
                            for t in range(3):
                                nc.tensor.matmul(
                                    p,
                                    lhsT=wob_sb[kt][:, (t * 2 + ct) * 128:(t * 2 + ct + 1) * 128],
                                    rhs=ctx[kt][:, ls * 512 + t: ls * 512 + t + 512],
                                    start=(n == 0), stop=(n == 5))
                                n += 1
                        osl = o_sb[:, ls * 512:(ls + 1) * 512]
                        i = ct * S * LS + s * LS + ls
                        nc.scalar.activation(
                            out=osl, in_=p, func=AF.Identity,
                            bias=bias["obb"][:, ct:ct + 1], scale=1.0,
                            accum_out=slots[1][:, i:i + 1])
                        sq_stat(1, osl, (2 + ct) * S * LS + s * LS + ls)

            def do_stats(br):
                statr = st.tile([128, 4], F32, tag=f"statr{br}", name=f"statr{br}")
                nc.vector.reduce_sum(
                    out=statr,
                    in_=slots[br].rearrange("p (g i) -> p g i", i=S * LS), axis=AXX)
                statp = st.tile([128, 4], F32, tag=f"statp{br}", name=f"statp{br}")
                nc.vector.tensor_scalar_mul(statp, statr, 1.0 / NSTAT)
                cc_in = dram.tile([128, 4], F32, tag=f"ccin{br}", name=f"ccin{br}")
                cc_out = dram.tile([128, 4], F32, tag=f"ccout{br}", name=f"ccout{br}")
                nc.sync.dma_start(out=cc_in, in_=statp)
                if os.environ.get("KERNEL_NO_CC"):
                    nc.sync.dma_start(out=cc_out, in_=cc_in)
                else:
                    nc.gpsimd.collective_compute(
                        "AllReduce", OP.add,
                        replica_groups=[list(range(NCORES))],
                        ins=[cc_in.opt()], outs=[cc_out.opt()])
                return cc_out

            def bn_coeffs(br, sg, gname, bname):
                mean = sg[:, 0:2]
                m2 = st.tile([128, CT], F32, tag=f"m2{br}", name=f"m2{br}")
                nc.vector.tensor_mul(out=m2, in0=mean, in1=mean)
                var = st.tile([128, CT], F32, tag=f"var{br}", name=f"var{br}")
                nc.vector.tensor_sub(out=var, in0=sg[:, 2:4], in1=m2)
                sd = st.tile([128, CT], F32, tag=f"sd{br}", name=f"sd{br}")
                nc.scalar.activation(out=sd, in_=var, func=AF.Sqrt, bias=eps_sb[:, 0:1], scale=1.0)
                rstd = st.tile([128, CT], F32, tag=f"rstd{br}", name=f"rstd{br}")
                nc.vector.reciprocal(out=rstd, in_=sd)
                A_ = st.tile([128, CT], F32, tag=f"A{br}", name=f"A{br}")
                nc.vector.tensor_mul(out=A_, in0=rstd, in1=bias[gname])
                mA = st.tile([128, CT], F32, tag=f"mA{br}", name=f"mA{br}")
                nc.vector.tensor_mul(out=mA, in0=mean, in1=A_)
                Bc = st.tile([128, CT], F32, tag=f"Bc{br}", name=f"Bc{br}")
                nc.vector.tensor_sub(out=Bc, in0=bias[bname], in1=mA)
                return A_, Bc

            def finalize(br, A_, Bc, gpsimd_only=False):
                for k, (s, ct) in enumerate((s, ct) for s in range(S) for ct in range(CT)):
                    tmp = bn.tile([128, L], F32, tag="bnt", name="bnt")
                    if gpsimd_only:
                        nc.gpsimd.tensor_scalar(
                            out=tmp, in0=o_tiles[(br, s, ct)],
                            scalar1=A_[:, ct:ct + 1], scalar2=Bc[:, ct:ct + 1],
                            op0=OP.mult, op1=OP.add)
                    elif k % 2 == 0:
                        nc.scalar.activation(
                            out=tmp, in_=o_tiles[(br, s, ct)], func=AF.Identity,
                            scale=A_[:, ct:ct + 1], bias=Bc[:, ct:ct + 1])
                    else:
                        nc.vector.tensor_scalar(
                            out=tmp, in0=o_tiles[(br, s, ct)],
                            scalar1=A_[:, ct:ct + 1], scalar2=Bc[:, ct:ct + 1],
                            op0=OP.mult, op1=OP.add)
                    res_t = bn.tile([128, L], F32, tag="bnr", name="bnr")
                    if gpsimd_only:
                        nc.gpsimd.tensor_add(
                            out=res_t, in0=tmp, in1=imm[s][ct][:, 1:L + 1])
                    elif k % 2 == 0:
                        nc.vector.tensor_add(
                            out=res_t, in0=tmp, in1=imm[s][ct][:, 1:L + 1])
                    else:
                        nc.gpsimd.tensor_add(
                            out=res_t, in0=tmp, in1=imm[s][ct][:, 1:L + 1])
                    nc.sync.dma_start(out=out_p[s, br, ct], in_=res_t)

            # ---- schedule ----
            gh_conv(0)
            gh_conv(1)
            v3_0 = a_attention(0)   # bv units for s0 fill the exp gaps
            v3_1 = a_attention(1)
            a_out_conv(0, v3_0)
            a_out_conv(1, v3_1)
            fill(99)                # any bv units not yet emitted
            cc_a = do_stats(0)
            sg_a = st.tile([128, 4], F32, tag="sg_a", name="sg_a")
            nc.sync.dma_start(out=sg_a, in_=cc_a)

            ctx0 = b_attention(0)
            b_out_conv(0, ctx0)
            # finalize a entirely on gpsimd: sits between AR-a and AR-b on the
            # gpsimd queue, so it runs as soon as AR-a lands, off the tail
            A_a, B_a = bn_coeffs(0, sg_a, "ga_a", "be_a")
            finalize(0, A_a, B_a, gpsimd_only=True)
            ctx1 = b_attention(1)
            b_out_conv(1, ctx1)

            # stats-b reduce + DMA first so AR-b can trigger promptly
            cc_b = do_stats(1)
            sg_b = st.tile([128, 4], F32, tag="sg_b", name="sg_b")
            nc.sync.dma_start(out=sg_b, in_=cc_b)
            A_b, B_b = bn_coeffs(1, sg_b, "ga_b", "be_b")
            finalize(1, A_b, B_b)

    nc.compile()
    return nc


def _get_nc():
    if not _NC_CACHE:
        _NC_CACHE.append(_build_nc())
    return _NC_CACHE[0]


def _prep_shared(inp):
    f32 = np.float32

    def smat(w, b):
        # [4, C]: rows 0..2 = taps of the 1-in-channel conv, row 3 = bias
        return np.concatenate(
            [np.asarray(w, f32)[:, 0, :].T, np.asarray(b, f32)[None, :]], axis=0)

    Wk_a = smat(inp["a_kw"], inp["a_kb"])
    Wv_a = smat(inp["a_vw"], inp["a_vb"])
    Wq_b = smat(inp["b_qw"], inp["b_qb"])

    m = {}
    # fused 4-row convs: g = Wk_a.(conv(img, a_qw)+a_qb), h = Wq_b.(conv+kb)
    qw2 = np.einsum('ic,cjt->ijt', Wk_a, np.asarray(inp["a_qw"], f32))
    kw2 = np.einsum('ic,cjt->ijt', Wq_b, np.asarray(inp["b_kw"], f32))
    wgh = np.zeros((128, 216), f32)
    for kt in range(CT):
        for t in range(3):
            b0 = (kt * 3 + t) * 36
            wgh[:, b0:b0 + 4] = qw2[:, kt * 128:(kt + 1) * 128, t].T
            wgh[:, b0 + 32:b0 + 36] = kw2[:, kt * 128:(kt + 1) * 128, t].T
    m["wgh"] = wgh.astype(NPBF)
    ghb = np.zeros((36, 1), f32)
    ghb[0:4, 0] = Wk_a @ np.asarray(inp["a_qb"], f32)
    ghb[32:36, 0] = Wq_b @ np.asarray(inp["b_kb"], f32)
    m["ghb"] = ghb

    # branch a out conv folded through Wv_a: rows 32t+i = sum_ci ow[:,ci,t]*Wv[i,ci]
    ow2 = np.einsum('oct,ic->tio', np.asarray(inp["a_ow"], f32), Wv_a)
    owa = np.zeros((68, 256), f32)
    for t in range(3):
        owa[32 * t:32 * t + 4] = ow2[t]
    m["owa"] = owa.astype(NPBF)

    m["wvb"] = np.ascontiguousarray(
        np.asarray(inp["b_vw"], f32).reshape(C, 2, 128, 3).transpose(1, 3, 2, 0)).astype(NPBF)
    m["vbb"] = np.ascontiguousarray(
        np.repeat(np.asarray(inp["b_vb"], f32)[None, :], 128, axis=0))
    m["wob"] = np.ascontiguousarray(
        np.asarray(inp["b_ow"], f32).reshape(2, 128, 2, 128, 3).transpose(2, 3, 4, 0, 1).reshape(2, 128, 768)).astype(NPBF)
    for dst, src in (("oba", "a_ob"), ("obb", "b_ob"),
                     ("ga_a", "a_g"), ("be_a", "a_beta"),
                     ("ga_b", "b_g"), ("be_b", "b_beta")):
        m[dst] = np.ascontiguousarray(np.asarray(inp[src], f32).reshape(2, 128).T)
    return m


def _core_maps(image, clinical, shared, ncores=NCORES):
    in_maps = []
    for core in range(ncores):
        m = dict(shared)
        sl = slice(core * S, (core + 1) * S)
        a = image[sl].reshape(S, CT, 128, L)
        pad = np.zeros((S, CT, 128, L + 2), np.float32)
        pad[..., 1:L + 1] = a
        m["imm"] = pad.astype(NPBF)
        c = clinical[sl][:, 0, :]
        im2 = np.zeros((S, 4, L), np.float32)
        im2[:, 0, 1:] = c[:, :L - 1]
        im2[:, 1, :] = c
        im2[:, 2, :L - 1] = c[:, 1:]
        im2[:, 3, :] = 1.0
        m["cli"] = im2.astype(NPBF)
        clit = np.zeros((S, 128, 288), np.float32)
        for s in range(S):
            for mt in range(8):
                blk = clit[s, :, mt * 36:mt * 36 + 36]
                blk[:, 0:4] = im2[s, :, mt * 128:(mt + 1) * 128].T
                blk[:, 32:36] = 1.0
        m["clit"] = clit.astype(NPBF)
        in_maps.append(m)
    return in_maps


def kernel(**inputs):
    inp = {k: np.asarray(v) for k, v in inputs.items()}
    nc = _get_nc()
    shared = _prep_shared(inp)
    image = inp["image"].astype(np.float32)
    clinical = inp["clinical"].astype(np.float32)
    in_maps = _core_maps(image, clinical, shared)
    res = run_bass_kernel_spmd(nc, in_maps, core_ids=list(range(NCORES)))
    outs = np.concatenate([res.results[i]["out"] for i in range(NCORES)], axis=0)
    return np.ascontiguousarray(outs.reshape(16, 512, L))
